# revision 61
# baseline (speedup 1.0000x reference)
"""Trainium2 Bass kernel for nn_AttentionBlock (B=4, L=S=1024, DIM=1024, NH=16).

Sharding: 8 cores = (batch b = core//2) x (head-half hh = core%2, 8 heads each).
Each core computes its batch's QKV projections restricted to its 512 feature
columns, attention for its 8 heads, and a partial output projection
(Wp row-slice); the host sums the two partials per batch.

Device layout is fully transposed ("T" = features/S on partitions) so no
on-device transposes are needed:
  qhT/khT (feat, L|S) from  Wslice.T @ xT ;  scoresT (S, L) = khT.T-slice @ qhT
  pos_bias+mask are merged host-side into epb = exp(pos_bias)*mask (pair-
  interleaved layout) and applied multiplicatively post-exp on the vector
  engine; softmax denominators ride a ones-column appended to V.
Schedule: pair 0's Q/K projections and the full V projection run dense
up-front; Q/K projections for pairs 1-3 are software-pipelined into the
attention loop as PE filler (3 instructions per attention unit, one pair
ahead), overlapping the scalar-engine-paced exp chain. Attention units run
lc-major so a single [65,1024] PV PSUM tile (both heads, ones-column
denominators) is live at a time, freeing PSUM banks for the filler
(scores 2x2 + PV 2 + filler 2x1 = 8 banks). Score/PV matmuls are trimmed
to the per-tile live row range [l0,l1) derived from the effective
causal+random mask (the epb multiply zeroes the skipped region exactly;
PSUM ring slots are memset once so the full-width exp stays finite).
Scores for both heads of a pair share one [128,1024] PSUM tile so
exp/multiply run as single wide instructions; softmax normalization is
batched per (pair, lc): Ln + Exp(-x) over the merged [1,1024] denominator
row, rank-1 PE broadcasts into PSUM quadrants, one copy, two muls.
Weights are host-prearranged into their consolidated SBUF tile layouts so
they load as single contiguous DMAs. Compute dtype bf16 (f32 PSUM), bf16
partial outputs. (fp8/DoubleRow projections were tried and reverted: the
dense fp8 burst trips a chip power throttle that slows the scalar/vector
engines ~20% for the rest of the kernel, a net loss.)
"""
import contextlib
import ctypes
import sys
import types

import numpy as np
import ml_dtypes

bf16 = ml_dtypes.bfloat16
fp8 = ml_dtypes.float8_e4m3

B, L, S, DIM, NH, DH = 4, 1024, 1024, 1024, 16, 64
NHC = 8           # heads per core
DIMC = 512        # feature columns per core
SCALE = 1.0 / np.sqrt(DH).astype(np.float32)

TRACE = False          # test.py flips this for profiling runs
TRACE_DIR = None
LAST_EXEC_NS = None


# ---------------------------------------------------------------- env setup
def _install_ntff_hook():
    if "antenv.axon_hooks" in sys.modules:
        return
    try:
        lib = ctypes.CDLL("/opt/axon/libaxon_pjrt.so")
        lib.axon_start_nrt_profile.argtypes = [
            ctypes.POINTER(ctypes.c_int64),
            ctypes.c_size_t,
        ]
        lib.axon_start_nrt_profile.restype = ctypes.c_int64
        lib.axon_stop_nrt_profile.argtypes = [ctypes.c_char_p]
        lib.axon_stop_nrt_profile.restype = ctypes.c_int64
    except OSError:
        return

    @contextlib.contextmanager
    def _hook(output_dir, device_ids):
        import jax

        jax.devices()
        if device_ids:
            ids = (ctypes.c_int64 * len(device_ids))(*device_ids)
            rc = lib.axon_start_nrt_profile(ids, len(device_ids))
        else:
            rc = lib.axon_start_nrt_profile(None, 0)
        if rc != 0:
            raise RuntimeError(f"axon_start_nrt_profile rc={rc}")
        try:
            yield
        finally:
            n = lib.axon_stop_nrt_profile(str(output_dir).encode())
            print(f"profile: {n} file(s) written to {output_dir}")

    mod = types.ModuleType("antenv.axon_hooks")
    mod.get_axon_ntff_profile_hook = lambda: _hook
    mod.set_axon_ntff_profile_hook = lambda h: None
    sys.modules["antenv.axon_hooks"] = mod


def _patch_tile_drain():
    from concourse import mybir
    from concourse.tile import TileContext, ScopedClock

    if getattr(TileContext, "_drain_split_patched", False):
        return

    def _drain_and_barrier(self, tick_clock, wait_clock):
        drain_inst = self.nc.sync.drain()
        wait_clock.add_sem_waits(
            drain_inst.ins, ScopedClock({None: tick_clock.global_clock})
        )
        waits = list(drain_inst.ins.sync_info.on_wait)
        if len(waits) > 1:
            drain_inst.ins.sync_info.on_wait = waits[:1]
            for w in waits[1:]:
                nop = self.nc.sync.nop()
                nop.ins.sync_info = mybir.SyncInfo(on_wait=[w], on_update=[])
        self.nc.all_engine_barrier()
        assert self.sems is not None
        popped = self.nc._tile_sem_poison_stack.pop()
        assert popped is self._sem_poison
        self.nc.clear_and_free_semaphores(list(self.sems.allocated().values()))
        self.nc.all_engine_barrier()

    TileContext._drain_and_barrier = _drain_and_barrier
    TileContext._drain_split_patched = True


def _split_multiwait_instructions(nc):
    """This container's walrus rejects >1 sync wait per instruction; hoist
    extras onto same-engine NOPs placed right before the instruction."""
    from concourse import mybir

    n_split = 0
    for fn in nc.m.functions:
        for bb in fn.blocks:
            out = []
            for inst in bb.instructions:
                si = inst.sync_info
                waits = list(si.on_wait) if si is not None else []
                if len(waits) > 1:
                    for w in waits[:-1]:
                        n_split += 1
                        out.append(
                            mybir.InstNoOp(
                                name=f"waitsplit-{n_split}-{inst.name}",
                                engine=inst.engine,
                                bass_nofuse=True,
                                sync_info=mybir.SyncInfo(on_wait=[w], on_update=[]),
                            )
                        )
                    si.on_wait = waits[-1:]
                out.append(inst)
            if n_split:
                bb.instructions = out


# ---------------------------------------------------------------- builder
_NC_CACHE = {}


def build_nc(use_bq=False, use_bk=False, use_bv=False, use_bp=False, trim=None):
    if trim is None:
        trim = tuple(tuple((0, 512) for _ in range(2)) for _ in range(8))
    live = tuple(tuple(t[1] > t[0] for t in row) for row in trim)
    key = (use_bq, use_bk, use_bv, use_bp, trim)
    if key in _NC_CACHE:
        return _NC_CACHE[key]
    _install_ntff_hook()
    _patch_tile_drain()
    import concourse.bass as bass
    import concourse.tile as tile
    from concourse import mybir

    dt = mybir.dt
    AF = mybir.ActivationFunctionType

    nc = bass.Bass("TRN2", target_bir_lowering=False, debug=False, num_devices=8)

    qT_d = nc.declare_dram_parameter("qT", (DIM, L), dt.bfloat16, isOutput=False)
    kT_d = nc.declare_dram_parameter("kT", (DIM, S), dt.bfloat16, isOutput=False)
    vT_d = nc.declare_dram_parameter("vT", (DIM, S), dt.bfloat16, isOutput=False)
    # host-prearranged big-tile layouts (see prep_inputs)
    wq_d = nc.declare_dram_parameter("wq", (128, 4096), dt.bfloat16, isOutput=False)
    wk_d = nc.declare_dram_parameter("wk", (128, 4096), dt.bfloat16, isOutput=False)
    wv_d = nc.declare_dram_parameter("wv", (128, 4096), dt.bfloat16, isOutput=False)
    wp_d = nc.declare_dram_parameter("wp", (128, 4096), dt.bfloat16, isOutput=False)
    pb_d = nc.declare_dram_parameter("pbT", (4 * S, 2 * L), dt.bfloat16, isOutput=False)
    bq_d = nc.declare_dram_parameter("bq", (1, DIMC), dt.float32, isOutput=False)
    bk_d = nc.declare_dram_parameter("bk", (1, DIMC), dt.float32, isOutput=False)
    bv_d = nc.declare_dram_parameter("bv", (1, DIMC), dt.float32, isOutput=False)
    bp_d = nc.declare_dram_parameter("bp", (128, 8), dt.float32, isOutput=False)
    out_d = nc.declare_dram_parameter("out", (DIM, L), dt.bfloat16, isOutput=True)

    live_sts = {lc: [st for st in range(8) if live[st][lc]] for lc in range(2)}
    first_live = {lc: live_sts[lc][0] for lc in range(2)}
    last_live = {lc: live_sts[lc][-1] for lc in range(2)}
    # per-pair attention unit list, lc-major so only one PV tile is live at a
    # time (frees PSUM banks for the projection filler)
    units = [(st, lc) for lc in range(2) for st in range(8) if live[st][lc]]
    n_lc0 = len(live_sts[0])

    with tile.TileContext(nc) as tc:
        with (
            tc.tile_pool(name="consts", bufs=1) as consts,
            tc.tile_pool(name="w", bufs=1) as wpool,
            tc.tile_pool(name="heads", bufs=1) as heads,
            tc.tile_pool(name="stage", bufs=2) as stage,
            tc.tile_pool(name="ostage", bufs=4) as ostage,
            tc.tile_pool(name="pb", bufs=2) as pbp,
        ):
            ones_t = consts.tile([128, 64], dt.bfloat16)
            nc.gpsimd.memset(ones_t[:], 1.0)
            warm = consts.tile([1, 64], dt.float32)
            nc.scalar.activation(warm[:], ones_t[0:1, :], AF.Exp)
            nc.scalar.activation(warm[:], ones_t[0:1, :], AF.Ln)
            # warm the gpsimd tensor_tensor ucode
            warm_g = consts.tile([128, 64], dt.bfloat16)
            nc.gpsimd.tensor_mul(warm_g[:], ones_t[:, 0:64], ones_t[:, 0:64])
            if use_bq:
                bq_t = consts.tile([1, DIMC], dt.float32)
                nc.sync.dma_start(bq_t[:], bq_d[:])
            if use_bk:
                bk_t = consts.tile([1, DIMC], dt.float32)
                nc.sync.dma_start(bk_t[:], bk_d[:])
            if use_bv:
                bv_t = consts.tile([1, DIMC], dt.float32)
                nc.sync.dma_start(bv_t[:], bv_d[:])
                ones_f = consts.tile([1, 128], dt.float32)
                nc.gpsimd.memset(ones_f[:], 1.0)
            if use_bq or use_bk:
                ones_r = consts.tile([1, 512], dt.float32)
                nc.gpsimd.memset(ones_r[:], 1.0)

            # host-prearranged weight tiles:
            #   wq/wk/wv[p, dtile*512+c] = W[dtile*128+p, c]
            #   wp[p, p4*1024+col]       = Wp[p4*128+p, col]
            wq_t = wpool.tile([128, 8 * DIMC], dt.bfloat16, name="wqb", tag="wqb")
            wk_t = wpool.tile([128, 8 * DIMC], dt.bfloat16, name="wkb", tag="wkb")
            wv_t = wpool.tile([128, 8 * DIMC], dt.bfloat16, name="wvb", tag="wvb")
            wp_t = wpool.tile([128, 4 * DIM], dt.bfloat16, name="wpb", tag="wpb")

            qh_t = [heads.tile([128, L], dt.bfloat16, name=f"qh{i}", tag=f"qh{i}") for i in range(4)]
            kh_t = [heads.tile([128, S], dt.bfloat16, name=f"kh{i}", tag=f"kh{i}") for i in range(4)]
            vh_t = [heads.tile([128, NHC * 65], dt.bfloat16, name=f"vh{i}", tag=f"vh{i}") for i in range(8)]
            oT_t = [heads.tile([128, L], dt.bfloat16, name=f"oT{i}", tag=f"oT{i}") for i in range(4)]

            def load_big(tile_ap, dram, rows, cols, chunks=1):
                # tile[:, a*cols + c] = dram[a*128 + p, c]
                n_a = rows // 128
                a_per = n_a // chunks
                for ch in range(chunks):
                    nc.sync.dma_start(
                        tile_ap[:, ch * a_per * cols : (ch + 1) * a_per * cols]
                        .rearrange("p (a c) -> p a c", c=cols),
                        dram[ch * a_per * 128 : (ch + 1) * a_per * 128, :]
                        .rearrange("(a p) c -> p a c", p=128),
                    )

            # -------- input loads: weight tiles are host-prearranged so they
            # load as single contiguous DMAs; x tiles are plain row slices
            xTp_cm = tc.tile_pool(name="xT", bufs=1)
            xTp = xTp_cm.__enter__()
            xvp_cm = tc.tile_pool(name="xvT", bufs=1)
            xvp = xvp_cm.__enter__()
            xq = [xTp.tile([128, 1024], dt.bfloat16, name=f"xq{i}", tag=f"xq{i}") for i in range(8)]
            xk = [xTp.tile([128, 1024], dt.bfloat16, name=f"xk{i}", tag=f"xk{i}") for i in range(8)]
            xv = [xvp.tile([128, 1024], dt.bfloat16, name=f"xv{i}", tag=f"xv{i}") for i in range(8)]

            nc.sync.dma_start(wq_t[:, 0:1024], wq_d[:, 0:1024])
            nc.sync.dma_start(xq[0][:], qT_d[0:128, :])
            nc.sync.dma_start(wq_t[:, 1024:4096], wq_d[:, 1024:4096])
            for i in range(1, 8):
                nc.sync.dma_start(xq[i][:], qT_d[i * 128 : (i + 1) * 128, :])
            nc.sync.dma_start(wk_t[:], wk_d[:])
            for i in range(8):
                nc.sync.dma_start(xk[i][:], kT_d[i * 128 : (i + 1) * 128, :])
            nc.sync.dma_start(wv_t[:], wv_d[:])
            for i in range(8):
                nc.sync.dma_start(xv[i][:], vT_d[i * 128 : (i + 1) * 128, :])

            # pos-bias (pre-merged exp(pb)*mask) tiles, per PAIR:
            # tile[p, st*2048 + lc*1024 + j*512 + li] = epb[h0+2p+j, l=lc*512+li, s=st*128+p]
            pb_tiles = {}

            def load_pb(p):
                pb_t = pbp.tile([128, 8 * 2048], dt.bfloat16, name=f"pb{p}", tag="pb")
                load_big(pb_t, pb_d[p * S : (p + 1) * S, :], S, 2048, chunks=4)
                pb_tiles[p] = pb_t

            load_pb(0)
            load_pb(1)
            bp_t = consts.tile([128, 8], dt.float32)
            # wp/bp loads are emitted inside pair 0's attention so they queue
            # behind the pre-attention stream instead of delaying it; pb for
            # pairs 2,3 loads lazily at the end of pair p-2's attention
            # (ring slot WAR needs the prior reads emitted first)

            # -------- projection emitters (Q/K as generators so they can be
            # interleaved into the attention stream as PE filler work)
            def qk_proj_gen(p, pool):
                """Yield after each instruction of pair p's Q/K projections."""
                for w_t, x_l, out_t, b_t, use_b in (
                    (wq_t, xq, qh_t[p], "bq", use_bq),
                    (wk_t, xk, kh_t[p], "bk", use_bk),
                ):
                    for lc in range(2):
                        lcs = slice(lc * 512, (lc + 1) * 512)
                        ps = pool.tile(
                            [128, 512], dt.float32, name=f"ps{p}_{lc}", tag="ps"
                        )
                        for dtile in range(8):
                            nc.tensor.matmul(
                                ps[:],
                                w_t[:, dtile * 512 + p * 128 : dtile * 512 + (p + 1) * 128],
                                x_l[dtile][:, lcs],
                                start=(dtile == 0),
                                stop=(dtile == 7) and not use_b,
                            )
                            yield
                        if use_b:
                            bt = bq_t if b_t == "bq" else bk_t
                            nc.tensor.matmul(
                                ps[:],
                                bt[0:1, p * 128 : (p + 1) * 128],
                                ones_r[0:1, 0:512],
                                start=False, stop=True,
                            )
                            yield
                        nc.vector.tensor_copy(out_t[:, lcs], ps[:])
                        yield

            # phase B: pair 0's Q/K then V, dense on the PE; Q/K for pairs
            # 1-3 are emitted later as PE filler inside the attention loop
            with tc.tile_pool(name="projB", bufs=4, space="PSUM") as projB:
                for _ in qk_proj_gen(0, projB):
                    pass
                for st in range(8):
                    psv = projB.tile([128, 512], dt.float32, name=f"psv{st}", tag="ps")
                    for dtile in range(8):
                        nc.tensor.matmul(
                            psv[:],
                            xv[dtile][:, st * 128 : (st + 1) * 128],
                            wv_t[:, dtile * 512 : (dtile + 1) * 512],
                            start=(dtile == 0),
                            stop=(dtile == 7) and not use_bv,
                        )
                    if use_bv:
                        nc.tensor.matmul(
                            psv[:],
                            ones_f[0:1, 0:128],
                            bv_t[:],
                            start=False, stop=True,
                        )
                    nc.gpsimd.memset(vh_t[st][:], 1.0)
                    nc.vector.tensor_copy(
                        vh_t[st].rearrange("p (h x) -> p h x", x=65)[:, :, 0:64],
                        psv[:].rearrange("p (h x) -> p h x", x=64),
                    )
            xvp_cm.__exit__(None, None, None)
            # ====== phase C: attention, deep-pipelined, head-merged tiles ======
            with (
                tc.tile_pool(name="scps", bufs=2, space="PSUM") as scps,
                tc.tile_pool(name="pvps", bufs=1, space="PSUM") as pvps,
                tc.tile_pool(name="fillps", bufs=2, space="PSUM") as fillps,
                tc.tile_pool(name="attn", bufs=8) as attnp,
            ):
                LAG = 2
                ats = {}
                pos = {}

                # one-time: make both scps ring slots finite so the full-width
                # exp over (causally) untouched score regions stays finite
                for mi in range(2):
                    mst = scps.tile([128, 1024], dt.float32, name=f"msc{mi}", tag="sc")
                    nc.vector.memset(mst[:], 0.0)

                def emit_scores(p, st, lc, mul_eng):
                    l0, l1 = trim[st][lc]
                    # one [128,1024] PSUM tile holds both heads' scores so the
                    # exp and epb-multiply run as single wide instructions
                    ps = scps.tile(
                        [128, 1024], dt.float32, name=f"sc{p}_{st}_{lc}", tag="sc"
                    )
                    for j in range(2):
                        jj = j * 64
                        nc.tensor.matmul(
                            ps[:, j * 512 + l0 : j * 512 + l1],
                            kh_t[p][jj : jj + 64, st * 128 : (st + 1) * 128],
                            qh_t[p][jj : jj + 64, lc * 512 + l0 : lc * 512 + l1],
                            start=True, stop=True,
                            tile_position=(jj, 0),
                        )
                    at = attnp.tile(
                        [128, 1024], dt.bfloat16, name=f"at{p}_{st}_{lc}", tag="attn"
                    )
                    cb = st * 2048 + lc * 1024
                    nc.scalar.activation(at[:], ps[:], AF.Exp)
                    mul_eng.tensor_mul(
                        at[:], at[:], pb_tiles[p][:, cb : cb + 1024]
                    )
                    ats[(p, st, lc)] = at

                def emit_pv(p, st, lc):
                    at = ats.pop((p, st, lc))
                    first = st == first_live[lc]
                    # first MM writes the full range so every PSUM element's
                    # has_written is set before any trimmed accumulation
                    l0, l1 = (0, 512) if first else trim[st][lc]
                    if first:
                        # both heads share one [65,1024] tile (j*512 halves) so
                        # the denominator rows form one [1,1024] AP for the norm
                        pos[lc] = pvps.tile(
                            [65, 1024], dt.float32, name=f"po{p}_{lc}", tag="pv"
                        )
                    for j in range(2):
                        h = 2 * p + j
                        nc.tensor.matmul(
                            pos[lc][:, j * 512 + l0 : j * 512 + l1],
                            vh_t[st][:, h * 65 : h * 65 + 65],
                            at[:, j * 512 + l0 : j * 512 + l1],
                            start=first,
                            stop=(st == last_live[lc]),
                        )

                def emit_norm_lc(p, lc):
                    # softmax denominators for both heads in single wide ops:
                    # ln, exp(-x) over [1,1024], rank-1 broadcast each head's
                    # reciprocal row to a PSUM quadrant, copy once, two muls
                    lcs = slice(lc * 512, (lc + 1) * 512)
                    po = pos[lc]
                    lnr = stage.tile([65, 1024], dt.float32, name=f"lnr{p}_{lc}", tag="lnr")
                    nc.scalar.activation(lnr[64:65, :], po[64:65, :], AF.Ln)
                    rec_bf = stage.tile([65, 1024], dt.bfloat16, name=f"rec{p}_{lc}", tag="rec")
                    nc.scalar.activation(rec_bf[64:65, :], lnr[64:65, :], AF.Exp, scale=-1.0)
                    # allocate from the filler ring, NOT the scores ring: a
                    # scores tile allocated after a pr would inherit a WAR on
                    # the norm's DVE copy and stall the whole exp chain
                    pr = fillps.tile([128, 512], dt.float32, name=f"pr{p}_{lc}", tag="ps")
                    for j in range(2):
                        nc.tensor.matmul(
                            pr[j * 64 : (j + 1) * 64, 0:512],
                            ones_t[64:65, 0:64],
                            rec_bf[64:65, j * 512 : (j + 1) * 512],
                            start=True, stop=True,
                            tile_position=(64, j * 64),
                        )
                    pr_sb = stage.tile([128, 512], dt.float32, name=f"prsb{p}_{lc}", tag="prsb")
                    nc.vector.tensor_copy(pr_sb[:], pr[0:128, 0:512])
                    for j in range(2):
                        nc.vector.tensor_mul(
                            oT_t[p][j * 64 : (j + 1) * 64, lcs],
                            po[0:64, j * 512 : (j + 1) * 512],
                            pr_sb[j * 64 : (j + 1) * 64, :],
                        )

                n_units = len(units)

                # Q/K projections for pairs 1-3, pumped a few instructions per
                # unit so the PE stays dense while the scalar engine paces the
                # exp chain; PE program order guarantees pair p's projections
                # land before pair p's first score matmul
                def filler_gen():
                    for fp in range(1, 4):
                        yield from qk_proj_gen(fp, fillps)

                fill = filler_gen()
                _DONE = object()

                def pump(n):
                    for _ in range(n):
                        if next(fill, _DONE) is _DONE:
                            return

                for p in range(4):
                    for i, (st, lc) in enumerate(units):
                        pump(3)
                        emit_scores(p, st, lc, nc.vector)
                        if p > 0 and i == 0:
                            emit_norm_lc(p - 1, 1)
                        if i == n_lc0 + LAG:
                            # all lc0 PVs are emitted; free the PV tile for lc1
                            emit_norm_lc(p, 0)
                        if i >= LAG:
                            emit_pv(p, *units[i - LAG])
                    if p == 0:
                        nc.sync.dma_start(wp_t[:], wp_d[:])
                        if use_bp:
                            nc.sync.dma_start(bp_t[:], bp_d[:])
                    for i in range(max(0, n_units - LAG), n_units):
                        emit_pv(p, *units[i])
                    if p < 2:
                        # prefetch pair p+2's merged pos-bias (slots of pair p,
                        # WAR now resolvable since pair p's reads are emitted)
                        load_pb(p + 2)
                pump(1000)
                emit_norm_lc(3, 1)
                assert not ats

            xTp_cm.__exit__(None, None, None)

            # ================= phase D: output projection =================
            with tc.tile_pool(name="finps", bufs=4, space="PSUM") as finps:
                for ot in range(8):
                    for lc in range(2):
                        lcs = slice(lc * 512, (lc + 1) * 512)
                        pf = finps.tile([128, 512], dt.float32, name=f"pf{ot}_{lc}", tag="fin")
                        for p4 in range(4):
                            nc.tensor.matmul(
                                pf[:],
                                wp_t[:, p4 * 1024 + ot * 128 : p4 * 1024 + (ot + 1) * 128],
                                oT_t[p4][:, lcs],
                                start=(p4 == 0),
                                stop=(p4 == 3),
                            )
                        f_sb = ostage.tile([128, 512], dt.bfloat16, name=f"fsb{ot}_{lc}", tag="fsb")
                        if use_bp:
                            nc.scalar.activation(
                                f_sb[:], pf[:], AF.Identity, bias=bp_t[:, ot : ot + 1]
                            )
                        else:
                            nc.vector.tensor_copy(f_sb[:], pf[:])
                        nc.sync.dma_start(
                            out_d[ot * 128 : (ot + 1) * 128, lcs], f_sb[:]
                        )

    _split_multiwait_instructions(nc)
    _NC_CACHE[key] = nc
    return nc


# ---------------------------------------------------------------- host side
def prep_inputs(inputs):
    """Shard + lay out the full inputs into 8 per-core input maps."""
    q = np.asarray(inputs["q"], np.float32)
    k = np.asarray(inputs["k"], np.float32)
    v = np.asarray(inputs["v"], np.float32)
    attn_mask = np.asarray(inputs["attn_mask"], bool)
    pos_bias = np.asarray(inputs["pos_bias"], np.float32)
    Wq = np.asarray(inputs["Wq"], np.float32)
    Wk = np.asarray(inputs["Wk"], np.float32)
    Wv = np.asarray(inputs["Wv"], np.float32)
    Wp = np.asarray(inputs["Wp"], np.float32)
    bq = np.asarray(inputs["bq"], np.float32)
    bk = np.asarray(inputs["bk"], np.float32)
    bv = np.asarray(inputs["bv"], np.float32)
    bp = np.asarray(inputs["bp"], np.float32)
    is_causal = int(np.asarray(inputs["is_causal"]))

    # effective mask: causal + row-any fix (matches the reference exactly)
    mask = attn_mask
    if is_causal:
        causal = np.tril(np.ones((L, L), bool))
        causal = np.pad(causal, ((0, 0), (S - L, 0)), constant_values=True)
        mask = mask & causal[None]
    row_any = mask.any(axis=-1, keepdims=True)
    mask = np.where(row_any, mask, True)  # (B, L, S)

    # merged multiplicative bias: exp(pos_bias) * mask, per-head (S, L) layout
    epb = np.exp(pos_bias)  # (B, NH, L, S)

    def big_w(w):
        # (1024, 512) -> [128, dtile*512 + c] consolidated tile layout
        return np.ascontiguousarray(
            w.reshape(8, 128, 512).transpose(1, 0, 2).reshape(128, 4096).astype(bf16)
        )

    in_maps = []
    for core in range(8):
        b, hh = core // 2, core % 2
        c0 = hh * DIMC
        h0 = hh * NHC
        wq_c = big_w(Wq[:, c0 : c0 + DIMC] * SCALE)
        wk_c = big_w(Wk[:, c0 : c0 + DIMC])
        wv_c = big_w(Wv[:, c0 : c0 + DIMC])
        wp_c = np.ascontiguousarray(
            Wp[c0 : c0 + DIMC, :].reshape(4, 128, 1024).transpose(1, 0, 2)
            .reshape(128, 4096).astype(bf16)
        )
        # pair-merged layout: pbT[p*S + s, lc*1024 + j*512 + li]
        #   = (exp(pos_bias)*mask)[b, h0+2p+j, lc*512+li, s]
        em = (epb[b, h0 : h0 + NHC] * mask[b][None]).astype(bf16)  # (8, L, S)
        epbT = (
            em.reshape(4, 2, 2, 512, S)
            .transpose(0, 4, 2, 1, 3)
            .reshape(4 * S, 2 * L)
        )
        in_maps.append(
            dict(
                qT=np.ascontiguousarray(q[b].T.astype(bf16)),
                kT=np.ascontiguousarray(k[b].T.astype(bf16)),
                vT=np.ascontiguousarray(v[b].T.astype(bf16)),
                wq=wq_c,
                wk=wk_c,
                wv=wv_c,
                wp=wp_c,
                pbT=np.ascontiguousarray(epbT),
                bq=np.ascontiguousarray((bq[c0 : c0 + DIMC] * SCALE)[None, :]),
                bk=np.ascontiguousarray(bk[c0 : c0 + DIMC][None, :]),
                bv=np.ascontiguousarray(bv[c0 : c0 + DIMC][None, :]),
                bp=(
                    np.ascontiguousarray(bp.reshape(8, 128).T)
                    if hh == 0
                    else np.zeros((128, 8), np.float32)
                ),
            )
        )
    # per-(S-tile, L-chunk) live row range: union of the effective mask over
    # batches. Rows outside [l0, l1) in a tile are fully masked for every
    # batch, so their scores/PV columns can be skipped (epb multiplies the
    # skipped region by exactly 0).
    mt = mask.any(axis=0)  # (L, S) union over batches
    trim = []
    for st in range(8):
        row = []
        for lc in range(2):
            sub = mt[lc * 512 : (lc + 1) * 512, st * 128 : (st + 1) * 128]
            liv = np.where(sub.any(axis=1))[0]
            if len(liv) == 0:
                row.append((0, 0))
            else:
                row.append((int(liv.min()), int(liv.max()) + 1))
        trim.append(tuple(row))
    return in_maps, tuple(trim)


def kernel(**inputs):
    global LAST_EXEC_NS
    from concourse.bass_utils import run_bass_kernel_spmd

    in_maps, trim = prep_inputs(inputs)
    nc = build_nc(
        use_bq=bool(np.any(np.asarray(inputs["bq"]))),
        use_bk=bool(np.any(np.asarray(inputs["bk"]))),
        use_bv=bool(np.any(np.asarray(inputs["bv"]))),
        use_bp=bool(np.any(np.asarray(inputs["bp"]))),
        trim=trim,
    )
    kwargs = {}
    if TRACE and TRACE_DIR:
        kwargs["tmpdir"] = TRACE_DIR
    res = run_bass_kernel_spmd(
        nc, in_maps, core_ids=list(range(8)), trace=TRACE, **kwargs
    )
    LAST_EXEC_NS = res.exec_time_ns
    outs = res.results
    out = np.empty((B, L, DIM), np.float32)
    for b in range(B):
        out[b] = (
            outs[2 * b]["out"].astype(np.float32)
            + outs[2 * b + 1]["out"].astype(np.float32)
        ).T
    return out



# revision 66
# speedup vs baseline: 1.1880x; 1.1880x over previous
"""Trainium2 Bass kernel for nn_AttentionBlock (B=4, L=S=1024, DIM=1024, NH=16).

Sharding: 8 cores = (batch b = core//2) x (head-half hh = core%2, 8 heads each).
Each core computes its batch's QKV projections restricted to its 512 feature
columns, attention for its 8 heads, and a partial output projection
(Wp row-slice); the host sums the two partials per batch.

Device layout is fully transposed ("T" = features/S on partitions) so no
on-device transposes are needed:
  qhT/khT (feat, L|S) from  Wslice.T @ xT ;  scoresT (S, L) = khT.T-slice @ qhT
  pos_bias+mask are merged host-side into epb = exp(pos_bias)*mask (pair-
  interleaved layout) and applied multiplicatively post-exp on the vector
  engine; softmax denominators ride a ones-column appended to V.
Schedule: pair 0's Q/K projections and the full V projection run dense
up-front; Q/K projections for pairs 1-3 are software-pipelined into the
attention loop as PE filler (3 instructions per attention unit, one pair
ahead), overlapping the scalar-engine-paced exp chain. Attention units run
lc-major so a single [65,1024] PV PSUM tile (both heads, ones-column
denominators) is live at a time, freeing PSUM banks for the filler
(scores 2x2 + PV 2 + filler 2x1 = 8 banks). Score/PV matmuls are trimmed
to the per-tile live row range [l0,l1) derived from the effective
causal+random mask (the epb multiply zeroes the skipped region exactly;
PSUM ring slots are memset once so the full-width exp stays finite).
Scores for both heads of a pair share one [128,1024] PSUM tile so
exp/multiply run as single wide instructions; softmax normalization is
batched per (pair, lc): Ln + Exp(-x) over the merged [1,1024] denominator
row, rank-1 PE broadcasts into PSUM quadrants, one copy, two muls.
Weights are host-prearranged into their consolidated SBUF tile layouts so
they load as single contiguous DMAs. Compute dtype bf16 (f32 PSUM), bf16
partial outputs. (fp8/DoubleRow projections were tried and reverted: the
dense fp8 burst trips a chip power throttle that slows the scalar/vector
engines ~20% for the rest of the kernel, a net loss.)
"""
import contextlib
import ctypes
import sys
import types

import numpy as np
import ml_dtypes

bf16 = ml_dtypes.bfloat16
fp8 = ml_dtypes.float8_e4m3

B, L, S, DIM, NH, DH = 4, 1024, 1024, 1024, 16, 64
NHC = 8           # heads per core
DIMC = 512        # feature columns per core
SCALE = 1.0 / np.sqrt(DH).astype(np.float32)

TRACE = False          # test.py flips this for profiling runs
TRACE_DIR = None
LAST_EXEC_NS = None


# ---------------------------------------------------------------- env setup
def _install_ntff_hook():
    if "antenv.axon_hooks" in sys.modules:
        return
    try:
        lib = ctypes.CDLL("/opt/axon/libaxon_pjrt.so")
        lib.axon_start_nrt_profile.argtypes = [
            ctypes.POINTER(ctypes.c_int64),
            ctypes.c_size_t,
        ]
        lib.axon_start_nrt_profile.restype = ctypes.c_int64
        lib.axon_stop_nrt_profile.argtypes = [ctypes.c_char_p]
        lib.axon_stop_nrt_profile.restype = ctypes.c_int64
    except OSError:
        return

    @contextlib.contextmanager
    def _hook(output_dir, device_ids):
        import jax

        jax.devices()
        if device_ids:
            ids = (ctypes.c_int64 * len(device_ids))(*device_ids)
            rc = lib.axon_start_nrt_profile(ids, len(device_ids))
        else:
            rc = lib.axon_start_nrt_profile(None, 0)
        if rc != 0:
            raise RuntimeError(f"axon_start_nrt_profile rc={rc}")
        try:
            yield
        finally:
            n = lib.axon_stop_nrt_profile(str(output_dir).encode())
            print(f"profile: {n} file(s) written to {output_dir}")

    mod = types.ModuleType("antenv.axon_hooks")
    mod.get_axon_ntff_profile_hook = lambda: _hook
    mod.set_axon_ntff_profile_hook = lambda h: None
    sys.modules["antenv.axon_hooks"] = mod


def _patch_tile_drain():
    from concourse import mybir
    from concourse.tile import TileContext, ScopedClock

    if getattr(TileContext, "_drain_split_patched", False):
        return

    def _drain_and_barrier(self, tick_clock, wait_clock):
        drain_inst = self.nc.sync.drain()
        wait_clock.add_sem_waits(
            drain_inst.ins, ScopedClock({None: tick_clock.global_clock})
        )
        waits = list(drain_inst.ins.sync_info.on_wait)
        if len(waits) > 1:
            drain_inst.ins.sync_info.on_wait = waits[:1]
            for w in waits[1:]:
                nop = self.nc.sync.nop()
                nop.ins.sync_info = mybir.SyncInfo(on_wait=[w], on_update=[])
        self.nc.all_engine_barrier()
        assert self.sems is not None
        popped = self.nc._tile_sem_poison_stack.pop()
        assert popped is self._sem_poison
        self.nc.clear_and_free_semaphores(list(self.sems.allocated().values()))
        self.nc.all_engine_barrier()

    TileContext._drain_and_barrier = _drain_and_barrier
    TileContext._drain_split_patched = True


def _split_multiwait_instructions(nc):
    """This container's walrus rejects >1 sync wait per instruction; hoist
    extras onto same-engine NOPs placed right before the instruction."""
    from concourse import mybir

    n_split = 0
    for fn in nc.m.functions:
        for bb in fn.blocks:
            out = []
            for inst in bb.instructions:
                si = inst.sync_info
                waits = list(si.on_wait) if si is not None else []
                if len(waits) > 1:
                    for w in waits[:-1]:
                        n_split += 1
                        out.append(
                            mybir.InstNoOp(
                                name=f"waitsplit-{n_split}-{inst.name}",
                                engine=inst.engine,
                                bass_nofuse=True,
                                sync_info=mybir.SyncInfo(on_wait=[w], on_update=[]),
                            )
                        )
                    si.on_wait = waits[-1:]
                out.append(inst)
            if n_split:
                bb.instructions = out


# ---------------------------------------------------------------- builder
_NC_CACHE = {}


def build_nc(use_bq=False, use_bk=False, use_bv=False, use_bp=False, trim=None):
    if trim is None:
        trim = tuple(tuple((0, 512) for _ in range(2)) for _ in range(8))
    live = tuple(tuple(t[1] > t[0] for t in row) for row in trim)
    key = (use_bq, use_bk, use_bv, use_bp, trim)
    if key in _NC_CACHE:
        return _NC_CACHE[key]
    _install_ntff_hook()
    _patch_tile_drain()
    import concourse.bass as bass
    import concourse.tile as tile
    from concourse import mybir

    dt = mybir.dt
    AF = mybir.ActivationFunctionType

    nc = bass.Bass("TRN2", target_bir_lowering=False, debug=False, num_devices=8)

    qT_d = nc.declare_dram_parameter("qT", (DIM, L), dt.bfloat16, isOutput=False)
    kT_d = nc.declare_dram_parameter("kT", (DIM, S), dt.bfloat16, isOutput=False)
    vT_d = nc.declare_dram_parameter("vT", (DIM, S), dt.bfloat16, isOutput=False)
    # host-prearranged big-tile layouts (see prep_inputs)
    wq_d = nc.declare_dram_parameter("wq", (128, 4096), dt.bfloat16, isOutput=False)
    wk_d = nc.declare_dram_parameter("wk", (128, 4096), dt.bfloat16, isOutput=False)
    wv_d = nc.declare_dram_parameter("wv", (128, 4096), dt.bfloat16, isOutput=False)
    wp_d = nc.declare_dram_parameter("wp", (128, 4096), dt.bfloat16, isOutput=False)
    pb_d = nc.declare_dram_parameter("pbT", (4 * S, 2 * L), dt.bfloat16, isOutput=False)
    bq_d = nc.declare_dram_parameter("bq", (1, DIMC), dt.float32, isOutput=False)
    bk_d = nc.declare_dram_parameter("bk", (1, DIMC), dt.float32, isOutput=False)
    bv_d = nc.declare_dram_parameter("bv", (1, DIMC), dt.float32, isOutput=False)
    bp_d = nc.declare_dram_parameter("bp", (128, 8), dt.float32, isOutput=False)
    out_d = nc.declare_dram_parameter("out", (DIM, L), dt.bfloat16, isOutput=True)

    live_sts = {lc: [st for st in range(8) if live[st][lc]] for lc in range(2)}
    first_live = {lc: live_sts[lc][0] for lc in range(2)}
    last_live = {lc: live_sts[lc][-1] for lc in range(2)}
    # per-pair attention unit list, lc-major so only one PV tile is live at a
    # time (frees PSUM banks for the projection filler)
    units = [(st, lc) for lc in range(2) for st in range(8) if live[st][lc]]
    n_lc0 = len(live_sts[0])

    with tile.TileContext(nc) as tc:
        with (
            tc.tile_pool(name="consts", bufs=1) as consts,
            tc.tile_pool(name="w", bufs=1) as wpool,
            tc.tile_pool(name="heads", bufs=1) as heads,
            tc.tile_pool(name="stage", bufs=2) as stage,
            tc.tile_pool(name="ostage", bufs=4) as ostage,
            tc.tile_pool(name="pb", bufs=2) as pbp,
        ):
            ones_t = consts.tile([128, 64], dt.bfloat16)
            nc.gpsimd.memset(ones_t[:], 1.0)
            warm = consts.tile([1, 64], dt.float32)
            nc.scalar.activation(warm[:], ones_t[0:1, :], AF.Exp)
            nc.scalar.activation(warm[:], ones_t[0:1, :], AF.Ln)
            # warm the gpsimd tensor_tensor ucode
            warm_g = consts.tile([128, 64], dt.bfloat16)
            nc.gpsimd.tensor_mul(warm_g[:], ones_t[:, 0:64], ones_t[:, 0:64])
            if use_bq:
                bq_t = consts.tile([1, DIMC], dt.float32)
                nc.sync.dma_start(bq_t[:], bq_d[:])
            if use_bk:
                bk_t = consts.tile([1, DIMC], dt.float32)
                nc.sync.dma_start(bk_t[:], bk_d[:])
            if use_bv:
                bv_t = consts.tile([1, DIMC], dt.float32)
                nc.sync.dma_start(bv_t[:], bv_d[:])
                ones_f = consts.tile([1, 128], dt.float32)
                nc.gpsimd.memset(ones_f[:], 1.0)
            if use_bq or use_bk:
                ones_r = consts.tile([1, 512], dt.float32)
                nc.gpsimd.memset(ones_r[:], 1.0)

            # host-prearranged weight tiles:
            #   wq/wk/wv[p, dtile*512+c] = W[dtile*128+p, c]
            #   wp[p, p4*1024+col]       = Wp[p4*128+p, col]
            wq_t = wpool.tile([128, 8 * DIMC], dt.bfloat16, name="wqb", tag="wqb")
            wk_t = wpool.tile([128, 8 * DIMC], dt.bfloat16, name="wkb", tag="wkb")
            wv_t = wpool.tile([128, 8 * DIMC], dt.bfloat16, name="wvb", tag="wvb")
            wp_t = wpool.tile([128, 4 * DIM], dt.bfloat16, name="wpb", tag="wpb")

            qh_t = [heads.tile([128, L], dt.bfloat16, name=f"qh{i}", tag=f"qh{i}") for i in range(4)]
            kh_t = [heads.tile([128, S], dt.bfloat16, name=f"kh{i}", tag=f"kh{i}") for i in range(4)]
            vh_t = [heads.tile([128, NHC * 65], dt.bfloat16, name=f"vh{i}", tag=f"vh{i}") for i in range(8)]
            oT_t = [heads.tile([128, L], dt.bfloat16, name=f"oT{i}", tag=f"oT{i}") for i in range(4)]

            def load_big(tile_ap, dram, rows, cols, chunks=1):
                # tile[:, a*cols + c] = dram[a*128 + p, c]
                n_a = rows // 128
                a_per = n_a // chunks
                for ch in range(chunks):
                    nc.sync.dma_start(
                        tile_ap[:, ch * a_per * cols : (ch + 1) * a_per * cols]
                        .rearrange("p (a c) -> p a c", c=cols),
                        dram[ch * a_per * 128 : (ch + 1) * a_per * 128, :]
                        .rearrange("(a p) c -> p a c", p=128),
                    )

            # -------- input loads: weight tiles are host-prearranged so they
            # load as single contiguous DMAs; x tiles are plain row slices
            xTp_cm = tc.tile_pool(name="xT", bufs=1)
            xTp = xTp_cm.__enter__()
            xvp_cm = tc.tile_pool(name="xvT", bufs=1)
            xvp = xvp_cm.__enter__()
            xq = [xTp.tile([128, 1024], dt.bfloat16, name=f"xq{i}", tag=f"xq{i}") for i in range(8)]
            xk = [xTp.tile([128, 1024], dt.bfloat16, name=f"xk{i}", tag=f"xk{i}") for i in range(8)]
            xv = [xvp.tile([128, 1024], dt.bfloat16, name=f"xv{i}", tag=f"xv{i}") for i in range(8)]

            nc.sync.dma_start(wq_t[:, 0:1024], wq_d[:, 0:1024])
            nc.sync.dma_start(xq[0][:], qT_d[0:128, :])
            nc.sync.dma_start(wq_t[:, 1024:4096], wq_d[:, 1024:4096])
            for i in range(1, 8):
                nc.sync.dma_start(xq[i][:], qT_d[i * 128 : (i + 1) * 128, :])
            nc.sync.dma_start(wv_t[:], wv_d[:])
            for i in range(8):
                nc.sync.dma_start(xv[i][:], vT_d[i * 128 : (i + 1) * 128, :])
            nc.sync.dma_start(wk_t[:], wk_d[:])
            for i in range(8):
                nc.sync.dma_start(xk[i][:], kT_d[i * 128 : (i + 1) * 128, :])

            # pos-bias (pre-merged exp(pb)*mask) tiles, per PAIR:
            # tile[p, st*2048 + lc*1024 + j*512 + li] = epb[h0+2p+j, l=lc*512+li, s=st*128+p]
            pb_tiles = {}

            def load_pb(p):
                pb_t = pbp.tile([128, 8 * 2048], dt.bfloat16, name=f"pb{p}", tag="pb")
                load_big(pb_t, pb_d[p * S : (p + 1) * S, :], S, 2048, chunks=4)
                pb_tiles[p] = pb_t

            load_pb(0)
            load_pb(1)
            bp_t = consts.tile([128, 8], dt.float32)
            # wp/bp loads are emitted inside pair 0's attention so they queue
            # behind the pre-attention stream instead of delaying it; pb for
            # pairs 2,3 loads lazily at the end of pair p-2's attention
            # (ring slot WAR needs the prior reads emitted first)

            # -------- projection emitters (Q/K as generators so they can be
            # interleaved into the attention stream as PE filler work)
            def qk_proj_gen(p, pool, parts=("q", "k")):
                """Yield after each instruction of pair p's Q/K projections."""
                streams = {
                    "q": (wq_t, xq, qh_t[p], "bq", use_bq),
                    "k": (wk_t, xk, kh_t[p], "bk", use_bk),
                }
                for w_t, x_l, out_t, b_t, use_b in (streams[s] for s in parts):
                    for lc in range(2):
                        lcs = slice(lc * 512, (lc + 1) * 512)
                        ps = pool.tile(
                            [128, 512], dt.float32, name=f"ps{p}_{lc}", tag="ps"
                        )
                        for dtile in range(8):
                            nc.tensor.matmul(
                                ps[:],
                                w_t[:, dtile * 512 + p * 128 : dtile * 512 + (p + 1) * 128],
                                x_l[dtile][:, lcs],
                                start=(dtile == 0),
                                stop=(dtile == 7) and not use_b,
                            )
                            yield
                        if use_b:
                            bt = bq_t if b_t == "bq" else bk_t
                            nc.tensor.matmul(
                                ps[:],
                                bt[0:1, p * 128 : (p + 1) * 128],
                                ones_r[0:1, 0:512],
                                start=False, stop=True,
                            )
                            yield
                        nc.vector.tensor_copy(out_t[:, lcs], ps[:])
                        yield

            # phase B: pair 0's Q proj, then V, then pair 0's K proj — in
            # DMA-arrival order (xq, xv, xk) so the PE never waits on loads;
            # Q/K for pairs 1-3 are emitted later as filler in the attention
            # loop
            with tc.tile_pool(name="projB", bufs=4, space="PSUM") as projB:
                for _ in qk_proj_gen(0, projB, parts=("q",)):
                    pass
                for st in range(8):
                    psv = projB.tile([128, 512], dt.float32, name=f"psv{st}", tag="ps")
                    for dtile in range(8):
                        nc.tensor.matmul(
                            psv[:],
                            xv[dtile][:, st * 128 : (st + 1) * 128],
                            wv_t[:, dtile * 512 : (dtile + 1) * 512],
                            start=(dtile == 0),
                            stop=(dtile == 7) and not use_bv,
                        )
                    if use_bv:
                        nc.tensor.matmul(
                            psv[:],
                            ones_f[0:1, 0:128],
                            bv_t[:],
                            start=False, stop=True,
                        )
                    nc.gpsimd.memset(vh_t[st][:], 1.0)
                    nc.vector.tensor_copy(
                        vh_t[st].rearrange("p (h x) -> p h x", x=65)[:, :, 0:64],
                        psv[:].rearrange("p (h x) -> p h x", x=64),
                    )
                for _ in qk_proj_gen(0, projB, parts=("k",)):
                    pass
            xvp_cm.__exit__(None, None, None)
            # ====== phase C: attention, deep-pipelined, head-merged tiles ======
            with (
                tc.tile_pool(name="scps", bufs=2, space="PSUM") as scps,
                tc.tile_pool(name="pvps", bufs=1, space="PSUM") as pvps,
                tc.tile_pool(name="fillps", bufs=2, space="PSUM") as fillps,
                tc.tile_pool(name="attn", bufs=8) as attnp,
            ):
                LAG = 2
                ats = {}
                pos = {}

                # one-time: make both scps ring slots finite so the full-width
                # exp over (causally) untouched score regions stays finite
                for mi in range(2):
                    mst = scps.tile([128, 1024], dt.float32, name=f"msc{mi}", tag="sc")
                    nc.vector.memset(mst[:], 0.0)

                def emit_scores(p, st, lc, mul_eng):
                    l0, l1 = trim[st][lc]
                    # one [128,1024] PSUM tile holds both heads' scores so the
                    # exp and epb-multiply run as single wide instructions
                    ps = scps.tile(
                        [128, 1024], dt.float32, name=f"sc{p}_{st}_{lc}", tag="sc"
                    )
                    for j in range(2):
                        jj = j * 64
                        nc.tensor.matmul(
                            ps[:, j * 512 + l0 : j * 512 + l1],
                            kh_t[p][jj : jj + 64, st * 128 : (st + 1) * 128],
                            qh_t[p][jj : jj + 64, lc * 512 + l0 : lc * 512 + l1],
                            start=True, stop=True,
                            tile_position=(jj, 0),
                        )
                    at = attnp.tile(
                        [128, 1024], dt.bfloat16, name=f"at{p}_{st}_{lc}", tag="attn"
                    )
                    cb = st * 2048 + lc * 1024
                    nc.scalar.activation(at[:], ps[:], AF.Exp)
                    mul_eng.tensor_mul(
                        at[:], at[:], pb_tiles[p][:, cb : cb + 1024]
                    )
                    ats[(p, st, lc)] = at

                def emit_pv(p, st, lc):
                    at = ats.pop((p, st, lc))
                    first = st == first_live[lc]
                    # first MM writes the full range so every PSUM element's
                    # has_written is set before any trimmed accumulation
                    l0, l1 = (0, 512) if first else trim[st][lc]
                    if first:
                        # both heads share one [65,1024] tile (j*512 halves) so
                        # the denominator rows form one [1,1024] AP for the norm
                        pos[lc] = pvps.tile(
                            [65, 1024], dt.float32, name=f"po{p}_{lc}", tag="pv"
                        )
                    for j in range(2):
                        h = 2 * p + j
                        nc.tensor.matmul(
                            pos[lc][:, j * 512 + l0 : j * 512 + l1],
                            vh_t[st][:, h * 65 : h * 65 + 65],
                            at[:, j * 512 + l0 : j * 512 + l1],
                            start=first,
                            stop=(st == last_live[lc]),
                        )

                def emit_norm_lc(p, lc):
                    # softmax denominators for both heads in single wide ops:
                    # ln, exp(-x) over [1,1024], rank-1 broadcast each head's
                    # reciprocal row to a PSUM quadrant, copy once, two muls
                    lcs = slice(lc * 512, (lc + 1) * 512)
                    po = pos[lc]
                    lnr = stage.tile([65, 1024], dt.float32, name=f"lnr{p}_{lc}", tag="lnr")
                    nc.scalar.activation(lnr[64:65, :], po[64:65, :], AF.Ln)
                    rec_bf = stage.tile([65, 1024], dt.bfloat16, name=f"rec{p}_{lc}", tag="rec")
                    nc.scalar.activation(rec_bf[64:65, :], lnr[64:65, :], AF.Exp, scale=-1.0)
                    pr = scps.tile([128, 1024], dt.float32, name=f"pr{p}_{lc}", tag="sc")
                    for j in range(2):
                        nc.tensor.matmul(
                            pr[j * 64 : (j + 1) * 64, 0:512],
                            ones_t[64:65, 0:64],
                            rec_bf[64:65, j * 512 : (j + 1) * 512],
                            start=True, stop=True,
                            tile_position=(64, j * 64),
                        )
                    pr_sb = stage.tile([128, 512], dt.float32, name=f"prsb{p}_{lc}", tag="prsb")
                    nc.vector.tensor_copy(pr_sb[:], pr[0:128, 0:512])
                    for j in range(2):
                        nc.vector.tensor_mul(
                            oT_t[p][j * 64 : (j + 1) * 64, lcs],
                            po[0:64, j * 512 : (j + 1) * 512],
                            pr_sb[j * 64 : (j + 1) * 64, :],
                        )

                n_units = len(units)

                # Q/K projections for pairs 1-3, pumped a few instructions per
                # unit so the PE stays dense while the scalar engine paces the
                # exp chain; PE program order guarantees pair p's projections
                # land before pair p's first score matmul
                def filler_gen():
                    for fp in range(1, 4):
                        yield from qk_proj_gen(fp, fillps)

                fill = filler_gen()
                _DONE = object()

                def pump(n):
                    for _ in range(n):
                        if next(fill, _DONE) is _DONE:
                            return

                for p in range(4):
                    for i, (st, lc) in enumerate(units):
                        pump(3)
                        emit_scores(p, st, lc, nc.vector)
                        if p > 0 and i == 0:
                            emit_norm_lc(p - 1, 1)
                        if i == n_lc0 + LAG:
                            # all lc0 PVs are emitted; free the PV tile for lc1
                            emit_norm_lc(p, 0)
                        if i >= LAG:
                            emit_pv(p, *units[i - LAG])
                    if p == 0:
                        nc.sync.dma_start(wp_t[:], wp_d[:])
                        if use_bp:
                            nc.sync.dma_start(bp_t[:], bp_d[:])
                    for i in range(max(0, n_units - LAG), n_units):
                        emit_pv(p, *units[i])
                    if p < 2:
                        # prefetch pair p+2's merged pos-bias (slots of pair p,
                        # WAR now resolvable since pair p's reads are emitted)
                        load_pb(p + 2)
                pump(1000)
                emit_norm_lc(3, 1)
                assert not ats

            xTp_cm.__exit__(None, None, None)

            # ================= phase D: output projection =================
            with tc.tile_pool(name="finps", bufs=4, space="PSUM") as finps:
                for ot in range(8):
                    for lc in range(2):
                        lcs = slice(lc * 512, (lc + 1) * 512)
                        pf = finps.tile([128, 512], dt.float32, name=f"pf{ot}_{lc}", tag="fin")
                        for p4 in range(4):
                            nc.tensor.matmul(
                                pf[:],
                                wp_t[:, p4 * 1024 + ot * 128 : p4 * 1024 + (ot + 1) * 128],
                                oT_t[p4][:, lcs],
                                start=(p4 == 0),
                                stop=(p4 == 3),
                            )
                        f_sb = ostage.tile([128, 512], dt.bfloat16, name=f"fsb{ot}_{lc}", tag="fsb")
                        if use_bp:
                            nc.scalar.activation(
                                f_sb[:], pf[:], AF.Identity, bias=bp_t[:, ot : ot + 1]
                            )
                        else:
                            nc.vector.tensor_copy(f_sb[:], pf[:])
                        nc.sync.dma_start(
                            out_d[ot * 128 : (ot + 1) * 128, lcs], f_sb[:]
                        )

    _split_multiwait_instructions(nc)
    _NC_CACHE[key] = nc
    return nc


# ---------------------------------------------------------------- host side
def prep_inputs(inputs):
    """Shard + lay out the full inputs into 8 per-core input maps."""
    q = np.asarray(inputs["q"], np.float32)
    k = np.asarray(inputs["k"], np.float32)
    v = np.asarray(inputs["v"], np.float32)
    attn_mask = np.asarray(inputs["attn_mask"], bool)
    pos_bias = np.asarray(inputs["pos_bias"], np.float32)
    Wq = np.asarray(inputs["Wq"], np.float32)
    Wk = np.asarray(inputs["Wk"], np.float32)
    Wv = np.asarray(inputs["Wv"], np.float32)
    Wp = np.asarray(inputs["Wp"], np.float32)
    bq = np.asarray(inputs["bq"], np.float32)
    bk = np.asarray(inputs["bk"], np.float32)
    bv = np.asarray(inputs["bv"], np.float32)
    bp = np.asarray(inputs["bp"], np.float32)
    is_causal = int(np.asarray(inputs["is_causal"]))

    # effective mask: causal + row-any fix (matches the reference exactly)
    mask = attn_mask
    if is_causal:
        causal = np.tril(np.ones((L, L), bool))
        causal = np.pad(causal, ((0, 0), (S - L, 0)), constant_values=True)
        mask = mask & causal[None]
    row_any = mask.any(axis=-1, keepdims=True)
    mask = np.where(row_any, mask, True)  # (B, L, S)

    # merged multiplicative bias: exp(pos_bias) * mask, per-head (S, L) layout
    epb = np.exp(pos_bias)  # (B, NH, L, S)

    def big_w(w):
        # (1024, 512) -> [128, dtile*512 + c] consolidated tile layout
        return np.ascontiguousarray(
            w.reshape(8, 128, 512).transpose(1, 0, 2).reshape(128, 4096).astype(bf16)
        )

    in_maps = []
    for core in range(8):
        b, hh = core // 2, core % 2
        c0 = hh * DIMC
        h0 = hh * NHC
        wq_c = big_w(Wq[:, c0 : c0 + DIMC] * SCALE)
        wk_c = big_w(Wk[:, c0 : c0 + DIMC])
        wv_c = big_w(Wv[:, c0 : c0 + DIMC])
        wp_c = np.ascontiguousarray(
            Wp[c0 : c0 + DIMC, :].reshape(4, 128, 1024).transpose(1, 0, 2)
            .reshape(128, 4096).astype(bf16)
        )
        # pair-merged layout: pbT[p*S + s, lc*1024 + j*512 + li]
        #   = (exp(pos_bias)*mask)[b, h0+2p+j, lc*512+li, s]
        em = (epb[b, h0 : h0 + NHC] * mask[b][None]).astype(bf16)  # (8, L, S)
        epbT = (
            em.reshape(4, 2, 2, 512, S)
            .transpose(0, 4, 2, 1, 3)
            .reshape(4 * S, 2 * L)
        )
        in_maps.append(
            dict(
                qT=np.ascontiguousarray(q[b].T.astype(bf16)),
                kT=np.ascontiguousarray(k[b].T.astype(bf16)),
                vT=np.ascontiguousarray(v[b].T.astype(bf16)),
                wq=wq_c,
                wk=wk_c,
                wv=wv_c,
                wp=wp_c,
                pbT=np.ascontiguousarray(epbT),
                bq=np.ascontiguousarray((bq[c0 : c0 + DIMC] * SCALE)[None, :]),
                bk=np.ascontiguousarray(bk[c0 : c0 + DIMC][None, :]),
                bv=np.ascontiguousarray(bv[c0 : c0 + DIMC][None, :]),
                bp=(
                    np.ascontiguousarray(bp.reshape(8, 128).T)
                    if hh == 0
                    else np.zeros((128, 8), np.float32)
                ),
            )
        )
    # per-(S-tile, L-chunk) live row range: union of the effective mask over
    # batches. Rows outside [l0, l1) in a tile are fully masked for every
    # batch, so their scores/PV columns can be skipped (epb multiplies the
    # skipped region by exactly 0).
    mt = mask.any(axis=0)  # (L, S) union over batches
    trim = []
    for st in range(8):
        row = []
        for lc in range(2):
            sub = mt[lc * 512 : (lc + 1) * 512, st * 128 : (st + 1) * 128]
            liv = np.where(sub.any(axis=1))[0]
            if len(liv) == 0:
                row.append((0, 0))
            else:
                row.append((int(liv.min()), int(liv.max()) + 1))
        trim.append(tuple(row))
    return in_maps, tuple(trim)


def kernel(**inputs):
    global LAST_EXEC_NS
    from concourse.bass_utils import run_bass_kernel_spmd

    in_maps, trim = prep_inputs(inputs)
    nc = build_nc(
        use_bq=bool(np.any(np.asarray(inputs["bq"]))),
        use_bk=bool(np.any(np.asarray(inputs["bk"]))),
        use_bv=bool(np.any(np.asarray(inputs["bv"]))),
        use_bp=bool(np.any(np.asarray(inputs["bp"]))),
        trim=trim,
    )
    kwargs = {}
    if TRACE and TRACE_DIR:
        kwargs["tmpdir"] = TRACE_DIR
    res = run_bass_kernel_spmd(
        nc, in_maps, core_ids=list(range(8)), trace=TRACE, **kwargs
    )
    LAST_EXEC_NS = res.exec_time_ns
    outs = res.results
    out = np.empty((B, L, DIM), np.float32)
    for b in range(B):
        out[b] = (
            outs[2 * b]["out"].astype(np.float32)
            + outs[2 * b + 1]["out"].astype(np.float32)
        ).T
    return out

